# revision 8
# baseline (speedup 1.0000x reference)
"""Trainium2 Bass kernel for quantized ConvBNReLU1D (pointwise conv k=1).

Reference computation (see problem spec):
    wq  = fake_quant_int8(W)  (per-tensor power-of-two scale)
    bq  = fake_quant_int8(b)
    y   = wq @ x + bq                  # [Cout,Cin] x [B,Cin,N]
    y   = y * inv + (beta - mean*inv)  # BN inference, inv = gamma*rsqrt(var+eps)
    y   = clip(round(relu(y)/as), 0, 255) * as   # QuantReLU

Strategy (v6 = v4 structure + warm PE + dual-ring loads):
  - Data-parallel over batch: 32 batches -> 4 per core on 8 cores.
  - x ships as fp16 (wq exact in fp16; only error is fp16 rounding of
    x, rel 0.0039 vs gate 2e-2). Output leaves as u8; host rebuilds
    y = u8 * act_scale.
  - 8 junk matmuls on memset tiles at t=0 keep the PE busy from ~0.5us
    so the HAM throttle flips to 2.4 GHz before real work; real MMs
    then stream warm (~216 ns/MM, LDWEIGHTS hidden).
  - Loads split across BOTH HWDGE rings: k0 tiles on the scalar ring,
    k1 tiles on sync, so batch 0's quarters arrive by ~3.8us and the
    remaining batches stay far ahead of the PE. No SWDGE (a third ring
    measurably degrades all rings).
  - Batch 0 in [128,1024] quarters per k (fast first feed); batches
    1-3 as whole [128,4096] tiles (8KB rows, ~290 GB/s).
  - Epilogue u8 = sat_u8(relu(psum*sv + bv)) alternates ScalarE / DVE
    per [128,1024] PSUM pair (exact RNE + clamp, probe-verified).
  - Stores on the sync ring; last batch mo-sequential with half stores
    to trim the drain tail.
"""

import os
import sys

import numpy as np

for _p in ("/opt/trn_rl_repo", "/root/.axon_site/_ro/trn_rl_repo"):
    if os.path.isdir(_p) and _p not in sys.path:
        sys.path.insert(0, _p)

from contextlib import ExitStack

import concourse.bacc as bacc
import concourse.tile as tile
from concourse import mybir
from concourse.bass import ts
from concourse.bass_utils import run_bass_kernel_spmd

F32 = mybir.dt.float32
F16 = mybir.dt.float16
U8 = mybir.dt.uint8
AF = mybir.ActivationFunctionType
ALU = mybir.AluOpType

N_CORES = 8
B, CIN, COUT, N = 32, 256, 256, 4096
B_SH = B // N_CORES  # batches per core
KC = CIN // 128      # K chunks
MC = COUT // 128     # output-channel chunks
NTILE = 512          # matmul free dim (one fp32 PSUM bank)
HW_ = 1024           # epilogue tile width (2 PSUM banks)
NH = N // HW_        # epilogue tiles per [128, N] row block
NJUNK = 5            # HAM warm-up matmuls

QMAX_W = 127.0
BN_EPS = 1e-5

_NC_CACHE = []
LAST_RESULTS = None  # BassKernelResults of the last run (for profiling)


def _build_nc():
    nc = bacc.Bacc("TRN2", target_bir_lowering=False)
    x_s = nc.declare_dram_parameter("x_s", [B_SH, CIN, N], F16, isOutput=False)
    # cc[:, :512] = wq.T chunks (k, mo); cc[:, 512:520] = f16-bitcast of
    # [sv_mo0, sv_mo1, bv_mo0, bv_mo1] f32 -- ONE DMA, few descriptors
    # (the early DMA phase is descriptor-rate limited).
    cc = nc.declare_dram_parameter("cc", [128, KC * MC * 128 + 8], F16, isOutput=False)
    y_u8 = nc.declare_dram_parameter("y_u8", [B_SH, COUT, N], U8, isOutput=True)

    with ExitStack() as ctx:
        tc = ctx.enter_context(tile.TileContext(nc))
        consts = ctx.enter_context(tc.tile_pool(name="consts", bufs=1))
        xqpool = ctx.enter_context(tc.tile_pool(name="xqpool", bufs=2 * KC * 2))
        xpool = ctx.enter_context(tc.tile_pool(name="xpool", bufs=KC * (B_SH - 2)))
        opool = ctx.enter_context(tc.tile_pool(name="opool", bufs=B_SH * MC))
        pspool = ctx.enter_context(tc.tile_pool(name="pspool", bufs=4, space="PSUM"))

        # --- junk tiles for PE warm-up (DVE memsets, ~0.5us) ---
        jw = consts.tile([128, 128], F16, tag="jw")
        nc.vector.memset(jw, 0.0)
        jx = consts.tile([128, NTILE], F16, tag="jx")
        nc.vector.memset(jx, 0.0)

        # --- batches 0,1 in [128,2048] halves: k0 scalar / k1 sync ---
        x_sb = {}  # (b, k) -> list of tiles covering [0, N)
        for b in (0, 1):
            for k in range(KC):
                x_sb[(b, k)] = []
        for b in (0, 1):
            for hh in range(2):
                for k, eng in ((0, nc.scalar), (1, nc.sync)):
                    xt = xqpool.tile(
                        [128, N // 2], F16, tag=f"xh{k}", name=f"xh{k}_{b}_{hh}"
                    )
                    eng.dma_start(
                        out=xt,
                        in_=x_s[b, k * 128 : (k + 1) * 128, ts(hh, N // 2)],
                    )
                    x_sb[(b, k)].append(xt)

        # --- combined consts: ONE DMA on the idle gpsimd ring ---
        sb = consts.tile([128, KC * MC * 128 + 8], F16, tag="cc")
        nc.gpsimd.dma_start(out=sb, in_=cc[:, :])
        w_sb = {
            (k, mo): sb[:, ts(2 * k + mo, 128)] for k in range(KC) for mo in range(MC)
        }
        svf = sb[:, KC * MC * 128 : KC * MC * 128 + 8].bitcast(F32)
        sv_sb = [svf[:, mo : mo + 1] for mo in range(MC)]
        bv_sb = [svf[:, MC + mo : MC + mo + 1] for mo in range(MC)]

        # --- ACT table warm-up (one-time ~1.3us ACT_TABLE_LOAD) ---
        wu_in = consts.tile([128, 8], F32, tag="wu_in")
        nc.vector.memset(wu_in, 0.0)
        wu_out = consts.tile([128, 8], U8, tag="wu_out")
        nc.scalar.activation(wu_out, wu_in, AF.Relu, bias=0.0, scale=1.0)

        # --- batches 2..3: whole [128,4096] tiles, k0 scalar / k1 sync ---
        for b in range(2, B_SH):
            for k in range(KC):
                xt = xpool.tile([128, N], F16, tag=f"x{k}", name=f"x{k}_{b}")
                eng = nc.scalar if k == 0 else nc.sync
                eng.dma_start(out=xt, in_=x_s[b, k * 128 : (k + 1) * 128, :])
                x_sb[(b, k)] = [xt]

        # --- junk matmuls: PE busy ~3.4us so HAM flips to 2.4 GHz ---
        jps = pspool.tile([128, HW_], F32, tag="ps")
        for _ in range(NJUNK):
            nc.tensor.matmul(jps[:, :NTILE], lhsT=jw, rhs=jx, start=True, stop=True)

        def rhs(b, k, h, j):
            parts = x_sb[(b, k)]
            col = h * HW_ + j * NTILE
            pw = N // len(parts)
            return parts[col // pw][:, col % pw : col % pw + NTILE]

        ep = 0  # alternates epilogue tiles between ScalarE and VectorE

        def epilogue(ot, ps, mo, h):
            nonlocal ep
            if ep % 2 == 0:
                nc.scalar.activation(
                    ot[:, ts(h, HW_)], ps, AF.Relu,
                    bias=bv_sb[mo], scale=sv_sb[mo],
                )
            else:
                nc.vector.tensor_scalar(
                    ot[:, ts(h, HW_)], ps, sv_sb[mo], bv_sb[mo],
                    ALU.mult, ALU.add,
                )
            ep += 1

        zig = [0]  # alternate k order tile-to-tile: ...k0,k1 | k1,k0...

        def mm_tile(ps, b, mo, h):
            order = (0, 1) if zig[0] % 2 == 0 else (1, 0)
            zig[0] += 1
            for ki, k in enumerate(order):
                for j in range(HW_ // NTILE):
                    nc.tensor.matmul(
                        ps[:, ts(j, NTILE)],
                        lhsT=w_sb[(k, mo)],
                        rhs=rhs(b, k, h, j),
                        start=(ki == 0),
                        stop=(ki == KC - 1),
                    )

        # Batches 0..B_SH-2: interleave mo0/mo1 per h. Stores (full row
        # blocks, 4 KB lines) on the sync ring.
        for b in range(B_SH - 1):
            ots = [
                opool.tile([128, N], U8, tag="o", name=f"o{b}_{mo}")
                for mo in range(MC)
            ]
            for h in range(NH):
                for mo in range(MC):
                    ps = pspool.tile([128, HW_], F32, tag="ps")
                    mm_tile(ps, b, mo, h)
                    epilogue(ots[mo], ps, mo, h)
                    if h == NH - 1:
                        nc.sync.dma_start(
                            out=y_u8[b, mo * 128 : (mo + 1) * 128, :],
                            in_=ots[mo],
                        )
        # Last batch: mo-sequential; final block stores in halves to
        # trim the drain tail.
        b = B_SH - 1
        for mo in range(MC):
            ot = opool.tile([128, N], U8, tag="o", name=f"o{b}_{mo}")
            for h in range(NH):
                ps = pspool.tile([128, HW_], F32, tag="ps")
                mm_tile(ps, b, mo, h)
                epilogue(ot, ps, mo, h)
                if mo == 0:
                    if h == NH - 1:
                        nc.sync.dma_start(
                            out=y_u8[b, :128, :], in_=ot
                        )
                else:
                    if h == NH // 2 - 1:
                        nc.sync.dma_start(
                            out=y_u8[b, 128:, : N // 2], in_=ot[:, : N // 2]
                        )
                    elif h == NH - 2:
                        nc.sync.dma_start(
                            out=y_u8[b, 128:, N // 2 : 3 * N // 4],
                            in_=ot[:, N // 2 : 3 * N // 4],
                        )
                    elif h == NH - 1:
                        nc.sync.dma_start(
                            out=y_u8[b, 128:, 3 * N // 4 :],
                            in_=ot[:, 3 * N // 4 :],
                        )
    nc.compile()
    return nc


def _host_fold(W, b, gamma, beta, running_mean, running_var, act_scale):
    """Fake-quant W/b exactly as the fp32 reference, fold BN + act scale."""
    f32 = np.float32

    def po2_scale(t):
        maxabs = np.maximum(np.max(np.abs(t)), f32(1e-12)).astype(f32)
        return np.exp2(np.ceil(np.log2(maxabs / f32(QMAX_W)))).astype(f32)

    def fake_quant(t, s):
        return (np.clip(np.round(t / s), -128.0, 127.0) * s).astype(f32)

    wq = fake_quant(W.astype(f32), po2_scale(W.astype(f32)))
    bq = fake_quant(b.astype(f32), po2_scale(b.astype(f32)))
    inv = (gamma.astype(f32) / np.sqrt(running_var.astype(f32) + f32(BN_EPS))).astype(f32)
    shift = (beta.astype(f32) - running_mean.astype(f32) * inv).astype(f32)
    a_s = f32(act_scale)
    sv = (inv / a_s).astype(f32)                    # per-channel matmul scale
    bv = ((bq * inv + shift) / a_s).astype(f32)     # per-channel bias
    wT = np.ascontiguousarray(wq.T).astype(np.float16)  # exact: int8 * po2
    return wT, sv, bv, a_s


def kernel(x, W, b, gamma, beta, running_mean, running_var, act_scale):
    global LAST_RESULTS
    if not _NC_CACHE:
        _NC_CACHE.append(_build_nc())
    nc = _NC_CACHE[0]

    wT, sv, bv, a_s = _host_fold(
        W, b, gamma, beta, running_mean, running_var, act_scale
    )
    cc = np.empty((128, KC * MC * 128 + 8), np.float16)
    for k in range(KC):
        for mo in range(MC):
            cc[:, (2 * k + mo) * 128 : (2 * k + mo + 1) * 128] = wT[
                k * 128 : (k + 1) * 128, mo * 128 : (mo + 1) * 128
            ]
    svbv4 = np.empty((128, 4), np.float32)
    for mo in range(MC):
        svbv4[:, mo] = sv[mo * 128 : (mo + 1) * 128]
        svbv4[:, MC + mo] = bv[mo * 128 : (mo + 1) * 128]
    cc[:, KC * MC * 128 :] = svbv4.view(np.float16)

    x_f16 = np.ascontiguousarray(np.asarray(x, dtype=np.float32)).astype(np.float16)

    in_maps = []
    for c in range(N_CORES):
        sl = slice(c * B_SH, (c + 1) * B_SH)
        in_maps.append({"x_s": x_f16[sl], "cc": cc})

    trace = bool(os.environ.get("KERNEL_TRACE"))
    try:
        res = run_bass_kernel_spmd(
            nc, in_maps, core_ids=list(range(N_CORES)), trace=trace
        )
    except Exception:
        if not trace:
            raise
        res = run_bass_kernel_spmd(
            nc, in_maps, core_ids=list(range(N_CORES)), trace=False
        )
    LAST_RESULTS = res
    u8 = np.concatenate([r["y_u8"] for r in res.results], axis=0)
    return u8.astype(np.float32) * a_s
